# revision 16
# baseline (speedup 1.0000x reference)
"""Trainium2 Bass kernel for MipRayMarcher2 (NeuS-style ray compositing).

Contract: kernel(**inputs) takes FULL unsharded numpy inputs, shards the
ray axis across 8 NeuronCores (fully data-parallel), runs one SPMD Bass
program per core, and reassembles the full outputs.

Shapes (hardcoded): B=4, R=16384, S=48 samples/ray.
Outputs (matching the reference tuple):
  composite_rgb   [B,R,3]
  composite_depth [B,R,1]
  weights         [B,R,S-1,1]
  composite_normal[B,R,3]

Per-core layout: 128 partitions x G rays/partition per tile, samples on the
free dim. Host-side prep (layout/dtype only): colors/real_normals are
transposed to channel-major, downcast to bf16 (their error only perturbs
their own composited outputs ~4e-3), and concatenated into one DMA tensor;
sdfs/depths/ray_directions are concatenated into another; normals stay fp32
ch-major (the weights output needs the cos path accurate).

Engine split (iterated via perfetto traces):
  GPSIMD  - the feed-forward front chain (nd, q, tc2, dl, a2, pp, e2) plus
            w/vd - it never waits on VectorE, so its in-order queue cannot
            stall the pipeline
  ScalarE - Relu (iter_cos) and Sigmoid (cdf)
  VectorE - alpha chain, fused segmented-scan transmittance, bf16 2x
            compositing multiplies, reductions, reciprocal_approx_fast
"""

import sys

for _p in ("/opt/trn_rl_repo", "/root/.axon_site/_ro/pypackages"):
    if _p not in sys.path:
        sys.path.insert(0, _p)

import ml_dtypes
import numpy as np

import concourse.bass as bass
import concourse.bacc as bacc
import concourse.tile as tile
from concourse import mybir
from concourse import bass_utils

# ---- problem constants --------------------------------------------------
B, R, S = 4, 16384, 48
SM = S - 1  # 47 mid samples
N_CORES = 8
RAYS = B * R                     # 65536
RAYS_PER_CORE = RAYS // N_CORES  # 8192
P = 128                          # partitions
G = 8                            # rays per partition per tile
TILE_RAYS = P * G                # 1024
N_TILES = RAYS_PER_CORE // TILE_RAYS  # 8

F32 = mybir.dt.float32
BF16 = mybir.dt.bfloat16
ALU = mybir.AluOpType
ACT = mybir.ActivationFunctionType
BF = ml_dtypes.bfloat16


def _build_program(k_half: float) -> bass.Bass:
    """Build the per-core Bass program. k_half = inv_std/2 (baked in)."""
    nc = bacc.Bacc("TRN2", target_bir_lowering=False, debug=False,
                   num_devices=N_CORES)

    # DRAM I/O (per-core shard, ray-flattened), host-concatenated:
    #   crn [rays, 6, S] bf16 : colors ch-major (0:3) + real_normals (3:6)
    #   sdd [rays, 99]  f32  : sdfs (0:48) + depths (48:96) + dirs (96:99)
    #   normals [rays, 3, S] f32 : normals ch-major
    crn_d = nc.dram_tensor("crn", [RAYS_PER_CORE, 6, S], BF16, kind="ExternalInput").ap()
    sdd_d = nc.dram_tensor("sdd", [RAYS_PER_CORE, 2 * S + 3], F32, kind="ExternalInput").ap()
    n_d = nc.dram_tensor("normals", [RAYS_PER_CORE, 3, S], F32, kind="ExternalInput").ap()

    w_d = nc.dram_tensor("wout", [RAYS_PER_CORE, SM], F32, kind="ExternalOutput").ap()
    # combo output: rgb (0:3) + depth (3) + normal (4:7)
    o_d = nc.dram_tensor("combo", [RAYS_PER_CORE, 7], F32, kind="ExternalOutput").ap()

    crn_r = crn_d.rearrange("(t p g) c s -> t p g c s", p=P, g=G)
    sdd_r = sdd_d.rearrange("(t p g) f -> t p g f", p=P, g=G)
    n_r = n_d.rearrange("(t p g) c s -> t p g c s", p=P, g=G)
    w_r = w_d.rearrange("(t p g) s -> t p g s", p=P, g=G)
    o_r = o_d.rearrange("(t p g) f -> t p g f", p=P, g=G)

    with tile.TileContext(nc) as tc:
        with (
            tc.tile_pool(name="consts", bufs=1) as consts,
            tc.tile_pool(name="ins", bufs=3) as ins,
            tc.tile_pool(name="tmp", bufs=3) as tmp,
            tc.tile_pool(name="outs", bufs=3) as outs,
        ):
            # segment-boundary mask for the fused transmittance scan:
            # 1.0 at s==0 of each group, 0 elsewhere
            bmask = consts.tile([P, G, S], F32)
            nc.vector.memset(bmask, 0.0)
            nc.vector.memset(bmask[:, :, 0:1], 1.0)
            eps_ap = consts.tile([P, 1], F32)     # 1e-5
            one_ap = consts.tile([P, 1], F32)     # 1 + 1e-10
            nc.vector.memset(eps_ap, 1e-5)
            nc.vector.memset(one_ap, 1.0000000001)

            for t in range(N_TILES):
                # ---- loads ----
                crn_t = ins.tile([P, G, 6, S], BF16, tag="crn")
                sdd_t = ins.tile([P, G, 2 * S + 3], F32, tag="sdd")
                n_t = ins.tile([P, G, 3, S], F32, tag="n")
                nc.sync.dma_start(out=crn_t, in_=crn_r[t])
                nc.sync.dma_start(out=sdd_t, in_=sdd_r[t])
                nc.sync.dma_start(out=n_t, in_=n_r[t])
                s_t = sdd_t[:, :, 0:S]
                d_t = sdd_t[:, :, S:2 * S]
                dir_t = sdd_t[:, :, 2 * S:2 * S + 3]
                c_t = crn_t[:, :, 0:3, :]
                rn_t = crn_t[:, :, 3:6, :]

                # ---- GPSIMD front chain (feed-forward, DMA-fed only) ----
                # q[s] = dot(dir, n[s]);  tc2 = q[s]+q[s+1] = 2*cos_mid
                nd = tmp.tile([P, G, 3, S], F32, tag="nd")
                for ch in range(3):
                    dir_b = dir_t[:, :, ch].unsqueeze(2).to_broadcast([P, G, S])
                    nc.gpsimd.tensor_tensor(nd[:, :, ch, :], n_t[:, :, ch, :],
                                            dir_b, ALU.mult)
                q = tmp.tile([P, G, S], F32, tag="q")
                nc.gpsimd.tensor_tensor(q, nd[:, :, 0, :], nd[:, :, 1, :], ALU.add)
                nc.gpsimd.tensor_tensor(q, q, nd[:, :, 2, :], ALU.add)
                tc2 = tmp.tile([P, G, SM], F32, tag="tc2")
                nc.gpsimd.tensor_tensor(tc2, q[:, :, 0:SM], q[:, :, 1:S], ALU.add)
                # m = relu(-0.5*tc2) = -iter_cos  (relu is positive-homogeneous)
                m = tmp.tile([P, G, SM], F32, tag="m")
                nc.scalar.activation(m, tc2, ACT.Relu, scale=-0.5)

                # deltas & 2*est sdfs: E+- = A2 +- m*delta
                dl = tmp.tile([P, G, SM], F32, tag="dl")
                nc.gpsimd.tensor_tensor(dl, d_t[:, :, 1:S], d_t[:, :, 0:SM], ALU.subtract)
                a2 = tmp.tile([P, G, SM], F32, tag="a2")
                nc.gpsimd.tensor_tensor(a2, s_t[:, :, 0:SM], s_t[:, :, 1:S], ALU.add)
                pp = tmp.tile([P, G, SM], F32, tag="pp")
                nc.gpsimd.tensor_tensor(pp, m, dl, ALU.mult)
                e2 = tmp.tile([P, G, 2, SM], F32, tag="e2")
                nc.gpsimd.tensor_tensor(e2[:, :, 0, :], a2, pp, ALU.add)       # 2*est_prev
                nc.gpsimd.tensor_tensor(e2[:, :, 1, :], a2, pp, ALU.subtract)  # 2*est_next
                cdf = tmp.tile([P, G, 2, SM], F32, tag="cdf")
                nc.scalar.activation(cdf, e2, ACT.Sigmoid, scale=float(k_half))

                # ---- alpha = clip((prev-next+1e-5)/(prev+1e-5), 0, 1) ----
                num2 = tmp.tile([P, G, SM], F32, tag="num2")
                nc.vector.scalar_tensor_tensor(num2, cdf[:, :, 0, :], 1e-5,
                                               cdf[:, :, 1, :], ALU.add, ALU.subtract)
                den = tmp.tile([P, G, SM], F32, tag="den")
                nc.scalar.activation(den, cdf[:, :, 0, :], ACT.Relu, bias=eps_ap)
                rden = tmp.tile([P, G, SM], F32, tag="rden")
                nc.vector.reciprocal_approx_fast(rden, den)
                alpha = tmp.tile([P, G, SM], F32, tag="alpha")
                nc.vector.tensor_tensor(alpha, num2, rden, ALU.mult)
                nc.vector.tensor_scalar(alpha, alpha, 1.0, 0.0, ALU.min, ALU.max)

                # ---- transmittance via ONE segmented scan over all groups:
                # z[g,0]=0, z[g,s]=om[g,s-1];  state = max(z*state, bmask)
                # bmask=1 at s==0 resets each segment exactly to 1.0f.
                z = tmp.tile([P, G, S], F32, tag="z")
                nc.vector.memset(z[:, :, 0:1], 0.0)
                nc.scalar.activation(z[:, :, 1:S], alpha, ACT.Relu,
                                     scale=-1.0, bias=one_ap)
                tx = tmp.tile([P, G, S], F32, tag="tx")
                nc.vector.tensor_tensor_scan(
                    tx.rearrange("p g s -> p (g s)"),
                    z.rearrange("p g s -> p (g s)"),
                    bmask.rearrange("p g s -> p (g s)"),
                    0.0, ALU.mult, ALU.max)

                # ---- w = alpha * tx;  wf padded with zeros at both ends
                wf = tmp.tile([P, G, SM + 2], F32, tag="wf")  # [0]=0, [1..47]=w, [48]=0
                nc.vector.memset(wf[:, :, 0:1], 0.0)
                nc.vector.memset(wf[:, :, SM + 1:SM + 2], 0.0)
                nc.vector.tensor_tensor(wf[:, :, 1:SM + 1], alpha, tx[:, :, 0:SM], ALU.mult)
                nc.sync.dma_start(out=w_r[t], in_=wf[:, :, 1:SM + 1])

                # ---- v[s] = w[s-1]+w[s]: composite = 0.5*sum v*x; sum v = 2*wt
                v = tmp.tile([P, G, S], F32, tag="v")
                nc.vector.tensor_tensor(v, wf[:, :, 0:S], wf[:, :, 1:S + 1], ALU.add)
                v_b = tmp.tile([P, G, S], BF16, tag="v_b")
                nc.scalar.copy(v_b, v)

                vcn = tmp.tile([P, G, 6, S], BF16, tag="vcn")
                vd = tmp.tile([P, G, S], F32, tag="vd")
                for ch in range(6):
                    nc.vector.tensor_tensor(vcn[:, :, ch, :], crn_t[:, :, ch, :],
                                            v_b, ALU.mult)
                nc.vector.tensor_tensor(vd, d_t, v, ALU.mult)

                cns = tmp.tile([P, G, 6], F32, tag="cns")
                dsum = tmp.tile([P, G], F32, tag="dsum")
                wt2 = tmp.tile([P, G], F32, tag="wt2")
                nc.vector.tensor_reduce(cns, vcn, mybir.AxisListType.X, ALU.add)
                nc.vector.tensor_reduce(dsum, vd, mybir.AxisListType.X, ALU.add)
                # sum v = 2 * sum w  (exactly)
                nc.vector.tensor_reduce(wt2, v, mybir.AxisListType.X, ALU.add)
                rgbs = cns[:, :, 0:3]
                nrms = cns[:, :, 3:6]

                # rwt2 = 1/(2*wt); wt2 in (~1e-5, 2], inside approx-fast range
                rwt2 = tmp.tile([P, G], F32, tag="rwt2")
                nc.vector.reciprocal_approx_fast(rwt2, wt2)

                # combo out: rgb (0:3) + depth (3) + normal (4:7)
                o_t = outs.tile([P, G, 7], F32, tag="o_t")
                nc.scalar.mul(o_t[:, :, 0:3], rgbs, 0.5)
                nc.vector.tensor_tensor(o_t[:, :, 3:4], dsum.unsqueeze(2),
                                        rwt2.unsqueeze(2), ALU.mult)
                rwt2_b = rwt2.unsqueeze(2).to_broadcast([P, G, 3])
                nc.vector.tensor_tensor(o_t[:, :, 4:7], nrms, rwt2_b, ALU.mult)
                nc.sync.dma_start(out=o_r[t], in_=o_t)

    nc.compile()
    return nc


_PROGRAM_CACHE: dict[float, bass.Bass] = {}


def _get_program(k_half: float) -> bass.Bass:
    if k_half not in _PROGRAM_CACHE:
        _PROGRAM_CACHE[k_half] = _build_program(k_half)
    return _PROGRAM_CACHE[k_half]


def kernel(colors, sdfs, depths, normals, ray_directions, real_normals,
           inv_std_param, _trace=False):
    colors = np.asarray(colors, dtype=np.float32)
    sdfs = np.asarray(sdfs, dtype=np.float32)
    depths = np.asarray(depths, dtype=np.float32)
    normals = np.asarray(normals, dtype=np.float32)
    ray_directions = np.asarray(ray_directions, dtype=np.float32)
    real_normals = np.asarray(real_normals, dtype=np.float32)

    p = np.float32(np.asarray(inv_std_param).reshape(()))
    inv_std = np.clip(np.exp(np.float32(10.0) * p), np.float32(1e-6), np.float32(1e6))
    k_half = float(np.float32(inv_std) * np.float32(0.5))

    nc = _get_program(k_half)

    # host prep (layout/dtype only): flatten rays, ch-major bf16, concat
    crn = np.empty((RAYS, 6, S), dtype=BF)
    crn[:, 0:3] = colors.reshape(RAYS, S, 3).transpose(0, 2, 1)
    crn[:, 3:6] = real_normals.reshape(RAYS, S, 3).transpose(0, 2, 1)
    sdd = np.empty((RAYS, 2 * S + 3), dtype=np.float32)
    sdd[:, 0:S] = sdfs.reshape(RAYS, S)
    sdd[:, S:2 * S] = depths.reshape(RAYS, S)
    sdd[:, 2 * S:] = ray_directions.reshape(RAYS, 3)
    nf = np.ascontiguousarray(normals.reshape(RAYS, S, 3).transpose(0, 2, 1))

    in_maps = []
    for k in range(N_CORES):
        lo, hi = k * RAYS_PER_CORE, (k + 1) * RAYS_PER_CORE
        in_maps.append({
            "crn": crn[lo:hi],
            "sdd": sdd[lo:hi],
            "normals": nf[lo:hi],
        })

    res = bass_utils.run_bass_kernel_spmd(
        nc, in_maps, core_ids=list(range(N_CORES)), trace=_trace)

    w = np.concatenate([res.results[k]["wout"] for k in range(N_CORES)], axis=0)
    combo = np.concatenate([res.results[k]["combo"] for k in range(N_CORES)], axis=0)
    rgb, dep, nrm = combo[:, 0:3], combo[:, 3], combo[:, 4:7]

    # faithful edge handling (no-ops for non-degenerate rays)
    dep = np.nan_to_num(dep, nan=np.inf)
    dep = np.clip(dep, depths.min(), depths.max())
    nrm = np.nan_to_num(nrm, nan=np.inf)
    nrm = np.clip(nrm, real_normals.min(), real_normals.max())

    out = (np.ascontiguousarray(rgb).reshape(B, R, 3).astype(np.float32),
           dep.reshape(B, R, 1).astype(np.float32),
           w.reshape(B, R, SM, 1).astype(np.float32),
           np.ascontiguousarray(nrm).reshape(B, R, 3).astype(np.float32))
    if _trace:
        return out, res
    return out


# revision 17
# speedup vs baseline: 1.0910x; 1.0910x over previous
"""Trainium2 Bass kernel for MipRayMarcher2 (NeuS-style ray compositing).

Contract: kernel(**inputs) takes FULL unsharded numpy inputs, shards the
ray axis across 8 NeuronCores (fully data-parallel), runs one SPMD Bass
program per core, and reassembles the full outputs.

Shapes (hardcoded): B=4, R=16384, S=48 samples/ray.
Outputs (matching the reference tuple):
  composite_rgb   [B,R,3]
  composite_depth [B,R,1]
  weights         [B,R,S-1,1]
  composite_normal[B,R,3]

Per-core layout: 128 partitions x G rays/partition per tile, samples on the
free dim. Host-side prep (layout/dtype only): colors/real_normals are
transposed to channel-major, downcast to bf16 (their error only perturbs
their own composited outputs ~4e-3), and concatenated into one DMA tensor;
sdfs/depths/ray_directions are concatenated into another; normals stay fp32
ch-major (the weights output needs the cos path accurate).

Engine split (iterated via perfetto traces):
  GPSIMD  - the feed-forward front chain (nd, q, tc2, dl, a2, pp, e2) plus
            w/vd - it never waits on VectorE, so its in-order queue cannot
            stall the pipeline
  ScalarE - Relu (iter_cos) and Sigmoid (cdf)
  VectorE - alpha chain, fused segmented-scan transmittance, bf16 2x
            compositing multiplies, reductions, reciprocal_approx_fast
"""

import sys

for _p in ("/opt/trn_rl_repo", "/root/.axon_site/_ro/pypackages"):
    if _p not in sys.path:
        sys.path.insert(0, _p)

import ml_dtypes
import numpy as np

import concourse.bass as bass
import concourse.bacc as bacc
import concourse.tile as tile
from concourse import mybir
from concourse import bass_utils

# ---- problem constants --------------------------------------------------
B, R, S = 4, 16384, 48
SM = S - 1  # 47 mid samples
N_CORES = 8
RAYS = B * R                     # 65536
RAYS_PER_CORE = RAYS // N_CORES  # 8192
P = 128                          # partitions
G = 8                            # rays per partition per tile
TILE_RAYS = P * G                # 1024
N_TILES = RAYS_PER_CORE // TILE_RAYS  # 8

F32 = mybir.dt.float32
BF16 = mybir.dt.bfloat16
ALU = mybir.AluOpType
ACT = mybir.ActivationFunctionType
BF = ml_dtypes.bfloat16


def _build_program(k_half: float) -> bass.Bass:
    """Build the per-core Bass program. k_half = inv_std/2 (baked in)."""
    nc = bacc.Bacc("TRN2", target_bir_lowering=False, debug=False,
                   num_devices=N_CORES)

    # DRAM I/O (per-core shard, ray-flattened), host-concatenated:
    #   crn [rays, 6, S] bf16 : colors ch-major (0:3) + real_normals (3:6)
    #   sdd [rays, 99]  f32  : sdfs (0:48) + depths (48:96) + dirs (96:99)
    #   normals [rays, 3, S] f32 : normals ch-major
    crn_d = nc.dram_tensor("crn", [RAYS_PER_CORE, 6, S], BF16, kind="ExternalInput").ap()
    sdd_d = nc.dram_tensor("sdd", [RAYS_PER_CORE, 2 * S + 3], F32, kind="ExternalInput").ap()
    n_d = nc.dram_tensor("normals", [RAYS_PER_CORE, 3, S], F32, kind="ExternalInput").ap()

    w_d = nc.dram_tensor("wout", [RAYS_PER_CORE, SM], F32, kind="ExternalOutput").ap()
    # combo output: rgb (0:3) + depth (3) + normal (4:7)
    o_d = nc.dram_tensor("combo", [RAYS_PER_CORE, 7], F32, kind="ExternalOutput").ap()

    crn_r = crn_d.rearrange("(t p g) c s -> t p g c s", p=P, g=G)
    sdd_r = sdd_d.rearrange("(t p g) f -> t p g f", p=P, g=G)
    n_r = n_d.rearrange("(t p g) c s -> t p g c s", p=P, g=G)
    w_r = w_d.rearrange("(t p g) s -> t p g s", p=P, g=G)
    o_r = o_d.rearrange("(t p g) f -> t p g f", p=P, g=G)

    with tile.TileContext(nc) as tc:
        with (
            tc.tile_pool(name="consts", bufs=1) as consts,
            tc.tile_pool(name="ins", bufs=3) as ins,
            tc.tile_pool(name="tmp", bufs=3) as tmp,
            tc.tile_pool(name="outs", bufs=3) as outs,
        ):
            # segment-boundary mask for the fused transmittance scan:
            # 1.0 at s==0 of each group, 0 elsewhere
            bmask = consts.tile([P, G, S], F32)
            nc.vector.memset(bmask, 0.0)
            nc.vector.memset(bmask[:, :, 0:1], 1.0)
            eps_ap = consts.tile([P, 1], F32)     # 1e-5
            one_ap = consts.tile([P, 1], F32)     # 1 + 1e-10
            nc.vector.memset(eps_ap, 1e-5)
            nc.vector.memset(one_ap, 1.0000000001)

            for t in range(N_TILES):
                # ---- loads ----
                crn_t = ins.tile([P, G, 6, S], BF16, tag="crn")
                sdd_t = ins.tile([P, G, 2 * S + 3], F32, tag="sdd")
                n_t = ins.tile([P, G, 3, S], F32, tag="n")
                nc.sync.dma_start(out=crn_t, in_=crn_r[t])
                nc.sync.dma_start(out=sdd_t, in_=sdd_r[t])
                nc.sync.dma_start(out=n_t, in_=n_r[t])
                s_t = sdd_t[:, :, 0:S]
                d_t = sdd_t[:, :, S:2 * S]
                dir_t = sdd_t[:, :, 2 * S:2 * S + 3]
                c_t = crn_t[:, :, 0:3, :]
                rn_t = crn_t[:, :, 3:6, :]

                # ---- GPSIMD front chain (feed-forward, DMA-fed only) ----
                # q[s] = dot(dir, n[s]);  tc2 = q[s]+q[s+1] = 2*cos_mid
                nd = tmp.tile([P, G, 3, S], F32, tag="nd")
                for ch in range(3):
                    dir_b = dir_t[:, :, ch].unsqueeze(2).to_broadcast([P, G, S])
                    nc.gpsimd.tensor_tensor(nd[:, :, ch, :], n_t[:, :, ch, :],
                                            dir_b, ALU.mult)
                q = tmp.tile([P, G, S], F32, tag="q")
                nc.gpsimd.tensor_tensor(q, nd[:, :, 0, :], nd[:, :, 1, :], ALU.add)
                nc.gpsimd.tensor_tensor(q, q, nd[:, :, 2, :], ALU.add)
                tc2 = tmp.tile([P, G, SM], F32, tag="tc2")
                nc.gpsimd.tensor_tensor(tc2, q[:, :, 0:SM], q[:, :, 1:S], ALU.add)
                # m = relu(-0.5*tc2) = -iter_cos  (relu is positive-homogeneous)
                m = tmp.tile([P, G, SM], F32, tag="m")
                nc.scalar.activation(m, tc2, ACT.Relu, scale=-0.5)

                # deltas & 2*est sdfs: E+- = A2 +- m*delta
                dl = tmp.tile([P, G, SM], F32, tag="dl")
                nc.gpsimd.tensor_tensor(dl, d_t[:, :, 1:S], d_t[:, :, 0:SM], ALU.subtract)
                a2 = tmp.tile([P, G, SM], F32, tag="a2")
                nc.gpsimd.tensor_tensor(a2, s_t[:, :, 0:SM], s_t[:, :, 1:S], ALU.add)
                pp = tmp.tile([P, G, SM], F32, tag="pp")
                nc.gpsimd.tensor_tensor(pp, m, dl, ALU.mult)
                e2 = tmp.tile([P, G, 2, SM], F32, tag="e2")
                nc.gpsimd.tensor_tensor(e2[:, :, 0, :], a2, pp, ALU.add)       # 2*est_prev
                nc.gpsimd.tensor_tensor(e2[:, :, 1, :], a2, pp, ALU.subtract)  # 2*est_next
                cdf = tmp.tile([P, G, 2, SM], F32, tag="cdf")
                nc.scalar.activation(cdf, e2, ACT.Sigmoid, scale=float(k_half))

                # ---- alpha = clip((prev-next+1e-5)/(prev+1e-5), 0, 1) ----
                num2 = tmp.tile([P, G, SM], F32, tag="num2")
                nc.vector.scalar_tensor_tensor(num2, cdf[:, :, 0, :], 1e-5,
                                               cdf[:, :, 1, :], ALU.add, ALU.subtract)
                den = tmp.tile([P, G, SM], F32, tag="den")
                nc.scalar.activation(den, cdf[:, :, 0, :], ACT.Relu, bias=eps_ap)
                rden = tmp.tile([P, G, SM], F32, tag="rden")
                nc.vector.reciprocal_approx_fast(rden, den)
                alpha = tmp.tile([P, G, SM], F32, tag="alpha")
                nc.vector.tensor_tensor(alpha, num2, rden, ALU.mult)
                nc.vector.tensor_scalar(alpha, alpha, 1.0, 0.0, ALU.min, ALU.max)

                # ---- transmittance via ONE segmented scan over all groups:
                # z[g,0]=0, z[g,s]=om[g,s-1];  state = max(z*state, bmask)
                # bmask=1 at s==0 resets each segment exactly to 1.0f.
                z = tmp.tile([P, G, S], F32, tag="z")
                nc.vector.memset(z[:, :, 0:1], 0.0)
                nc.vector.tensor_scalar(z[:, :, 1:S], alpha, -1.0, 1.0000000001,
                                        ALU.mult, ALU.add)
                tx = tmp.tile([P, G, S], F32, tag="tx")
                nc.vector.tensor_tensor_scan(
                    tx.rearrange("p g s -> p (g s)"),
                    z.rearrange("p g s -> p (g s)"),
                    bmask.rearrange("p g s -> p (g s)"),
                    0.0, ALU.mult, ALU.max)

                # ---- w = alpha * tx;  wf padded with zeros at both ends
                wf = tmp.tile([P, G, SM + 2], F32, tag="wf")  # [0]=0, [1..47]=w, [48]=0
                nc.vector.memset(wf[:, :, 0:1], 0.0)
                nc.vector.memset(wf[:, :, SM + 1:SM + 2], 0.0)
                nc.vector.tensor_tensor(wf[:, :, 1:SM + 1], alpha, tx[:, :, 0:SM], ALU.mult)
                nc.sync.dma_start(out=w_r[t], in_=wf[:, :, 1:SM + 1])

                # ---- v[s] = w[s-1]+w[s]: composite = 0.5*sum v*x; sum v = 2*wt
                v = tmp.tile([P, G, S], F32, tag="v")
                nc.vector.tensor_tensor(v, wf[:, :, 0:S], wf[:, :, 1:S + 1], ALU.add)
                v_b = tmp.tile([P, G, S], BF16, tag="v_b")
                nc.vector.tensor_copy(v_b, v)

                vcn = tmp.tile([P, G, 6, S], BF16, tag="vcn")
                vd = tmp.tile([P, G, S], F32, tag="vd")
                for ch in range(6):
                    nc.vector.tensor_tensor(vcn[:, :, ch, :], crn_t[:, :, ch, :],
                                            v_b, ALU.mult)
                nc.vector.tensor_tensor(vd, d_t, v, ALU.mult)

                cns = tmp.tile([P, G, 6], F32, tag="cns")
                dsum = tmp.tile([P, G], F32, tag="dsum")
                wt2 = tmp.tile([P, G], F32, tag="wt2")
                nc.vector.tensor_reduce(cns, vcn, mybir.AxisListType.X, ALU.add)
                nc.vector.tensor_reduce(dsum, vd, mybir.AxisListType.X, ALU.add)
                # sum v = 2 * sum w  (exactly)
                nc.vector.tensor_reduce(wt2, v, mybir.AxisListType.X, ALU.add)
                rgbs = cns[:, :, 0:3]
                nrms = cns[:, :, 3:6]

                # rwt2 = 1/(2*wt); wt2 in (~1e-5, 2], inside approx-fast range
                rwt2 = tmp.tile([P, G], F32, tag="rwt2")
                nc.vector.reciprocal_approx_fast(rwt2, wt2)

                # combo out: rgb (0:3) + depth (3) + normal (4:7)
                o_t = outs.tile([P, G, 7], F32, tag="o_t")
                nc.scalar.mul(o_t[:, :, 0:3], rgbs, 0.5)
                nc.vector.tensor_tensor(o_t[:, :, 3:4], dsum.unsqueeze(2),
                                        rwt2.unsqueeze(2), ALU.mult)
                rwt2_b = rwt2.unsqueeze(2).to_broadcast([P, G, 3])
                nc.vector.tensor_tensor(o_t[:, :, 4:7], nrms, rwt2_b, ALU.mult)
                nc.sync.dma_start(out=o_r[t], in_=o_t)

    nc.compile()
    return nc


_PROGRAM_CACHE: dict[float, bass.Bass] = {}


def _get_program(k_half: float) -> bass.Bass:
    if k_half not in _PROGRAM_CACHE:
        _PROGRAM_CACHE[k_half] = _build_program(k_half)
    return _PROGRAM_CACHE[k_half]


def kernel(colors, sdfs, depths, normals, ray_directions, real_normals,
           inv_std_param, _trace=False):
    colors = np.asarray(colors, dtype=np.float32)
    sdfs = np.asarray(sdfs, dtype=np.float32)
    depths = np.asarray(depths, dtype=np.float32)
    normals = np.asarray(normals, dtype=np.float32)
    ray_directions = np.asarray(ray_directions, dtype=np.float32)
    real_normals = np.asarray(real_normals, dtype=np.float32)

    p = np.float32(np.asarray(inv_std_param).reshape(()))
    inv_std = np.clip(np.exp(np.float32(10.0) * p), np.float32(1e-6), np.float32(1e6))
    k_half = float(np.float32(inv_std) * np.float32(0.5))

    nc = _get_program(k_half)

    # host prep (layout/dtype only): flatten rays, ch-major bf16, concat
    crn = np.empty((RAYS, 6, S), dtype=BF)
    crn[:, 0:3] = colors.reshape(RAYS, S, 3).transpose(0, 2, 1)
    crn[:, 3:6] = real_normals.reshape(RAYS, S, 3).transpose(0, 2, 1)
    sdd = np.empty((RAYS, 2 * S + 3), dtype=np.float32)
    sdd[:, 0:S] = sdfs.reshape(RAYS, S)
    sdd[:, S:2 * S] = depths.reshape(RAYS, S)
    sdd[:, 2 * S:] = ray_directions.reshape(RAYS, 3)
    nf = np.ascontiguousarray(normals.reshape(RAYS, S, 3).transpose(0, 2, 1))

    in_maps = []
    for k in range(N_CORES):
        lo, hi = k * RAYS_PER_CORE, (k + 1) * RAYS_PER_CORE
        in_maps.append({
            "crn": crn[lo:hi],
            "sdd": sdd[lo:hi],
            "normals": nf[lo:hi],
        })

    res = bass_utils.run_bass_kernel_spmd(
        nc, in_maps, core_ids=list(range(N_CORES)), trace=_trace)

    w = np.concatenate([res.results[k]["wout"] for k in range(N_CORES)], axis=0)
    combo = np.concatenate([res.results[k]["combo"] for k in range(N_CORES)], axis=0)
    rgb, dep, nrm = combo[:, 0:3], combo[:, 3], combo[:, 4:7]

    # faithful edge handling (no-ops for non-degenerate rays)
    dep = np.nan_to_num(dep, nan=np.inf)
    dep = np.clip(dep, depths.min(), depths.max())
    nrm = np.nan_to_num(nrm, nan=np.inf)
    nrm = np.clip(nrm, real_normals.min(), real_normals.max())

    out = (np.ascontiguousarray(rgb).reshape(B, R, 3).astype(np.float32),
           dep.reshape(B, R, 1).astype(np.float32),
           w.reshape(B, R, SM, 1).astype(np.float32),
           np.ascontiguousarray(nrm).reshape(B, R, 3).astype(np.float32))
    if _trace:
        return out, res
    return out


# revision 19
# speedup vs baseline: 1.1006x; 1.0088x over previous
"""Trainium2 Bass kernel for MipRayMarcher2 (NeuS-style ray compositing).

Contract: kernel(**inputs) takes FULL unsharded numpy inputs, shards the
ray axis across 8 NeuronCores (fully data-parallel), runs one SPMD Bass
program per core, and reassembles the full outputs.

Shapes (hardcoded): B=4, R=16384, S=48 samples/ray.
Outputs (matching the reference tuple):
  composite_rgb   [B,R,3]
  composite_depth [B,R,1]
  weights         [B,R,S-1,1]
  composite_normal[B,R,3]

Per-core layout: 128 partitions x G rays/partition per tile, samples on the
free dim. Host-side prep (layout/dtype only): colors/real_normals are
transposed to channel-major, downcast to bf16 (their error only perturbs
their own composited outputs ~4e-3), and concatenated into one DMA tensor;
sdfs/depths/ray_directions are concatenated into another; normals stay fp32
ch-major (the weights output needs the cos path accurate).

Engine split (iterated via perfetto traces):
  GPSIMD  - the feed-forward front chain (nd, q, tc2, dl, a2, pp, e2) plus
            w/vd - it never waits on VectorE, so its in-order queue cannot
            stall the pipeline
  ScalarE - Relu (iter_cos) and Sigmoid (cdf)
  VectorE - alpha chain, fused segmented-scan transmittance, bf16 2x
            compositing multiplies, reductions, reciprocal_approx_fast
"""

import sys

for _p in ("/opt/trn_rl_repo", "/root/.axon_site/_ro/pypackages"):
    if _p not in sys.path:
        sys.path.insert(0, _p)

import ml_dtypes
import numpy as np

import concourse.bass as bass
import concourse.bacc as bacc
import concourse.tile as tile
from concourse import mybir
from concourse import bass_utils

# ---- problem constants --------------------------------------------------
B, R, S = 4, 16384, 48
SM = S - 1  # 47 mid samples
N_CORES = 8
RAYS = B * R                     # 65536
RAYS_PER_CORE = RAYS // N_CORES  # 8192
P = 128                          # partitions
G = 8                            # rays per partition per tile
TILE_RAYS = P * G                # 1024
N_TILES = RAYS_PER_CORE // TILE_RAYS  # 8

F32 = mybir.dt.float32
BF16 = mybir.dt.bfloat16
ALU = mybir.AluOpType
ACT = mybir.ActivationFunctionType
BF = ml_dtypes.bfloat16


def _build_program(k_half: float) -> bass.Bass:
    """Build the per-core Bass program. k_half = inv_std/2 (baked in)."""
    nc = bacc.Bacc("TRN2", target_bir_lowering=False, debug=False,
                   num_devices=N_CORES)

    # DRAM I/O (per-core shard, ray-flattened), host-concatenated:
    #   crn [rays, 6, S] bf16 : colors ch-major (0:3) + real_normals (3:6)
    #   sdd [rays, 99]  f32  : sdfs (0:48) + depths (48:96) + dirs (96:99)
    #   normals [rays, 3, S] f32 : normals ch-major
    crn_d = nc.dram_tensor("crn", [RAYS_PER_CORE, 6, S], BF16, kind="ExternalInput").ap()
    sdd_d = nc.dram_tensor("sdd", [RAYS_PER_CORE, 2 * S + 3], F32, kind="ExternalInput").ap()
    n_d = nc.dram_tensor("normals", [RAYS_PER_CORE, 3, S], F32, kind="ExternalInput").ap()

    w_d = nc.dram_tensor("wout", [RAYS_PER_CORE, SM], F32, kind="ExternalOutput").ap()
    # combo output: rgb (0:3) + depth (3) + normal (4:7)
    o_d = nc.dram_tensor("combo", [RAYS_PER_CORE, 7], F32, kind="ExternalOutput").ap()

    crn_r = crn_d.rearrange("(t p g) c s -> t p g c s", p=P, g=G)
    sdd_r = sdd_d.rearrange("(t p g) f -> t p g f", p=P, g=G)
    n_r = n_d.rearrange("(t p g) c s -> t p g c s", p=P, g=G)
    w_r = w_d.rearrange("(t p g) s -> t p g s", p=P, g=G)
    o_r = o_d.rearrange("(t p g) f -> t p g f", p=P, g=G)

    with tile.TileContext(nc) as tc:
        with (
            tc.tile_pool(name="consts", bufs=1) as consts,
            tc.tile_pool(name="ins", bufs=4) as ins,
            tc.tile_pool(name="tmp", bufs=3) as tmp,
            tc.tile_pool(name="outs", bufs=3) as outs,
        ):
            # segment-boundary mask for the fused transmittance scan:
            # 1.0 at s==0 of each group, 0 elsewhere
            bmask = consts.tile([P, G, S], F32)
            nc.vector.memset(bmask, 0.0)
            nc.vector.memset(bmask[:, :, 0:1], 1.0)
            eps_ap = consts.tile([P, 1], F32)     # 1e-5
            one_ap = consts.tile([P, 1], F32)     # 1 + 1e-10
            nc.vector.memset(eps_ap, 1e-5)
            nc.vector.memset(one_ap, 1.0000000001)

            for t in range(N_TILES):
                # ---- loads ----
                crn_t = ins.tile([P, G, 6, S], BF16, tag="crn")
                sdd_t = ins.tile([P, G, 2 * S + 3], F32, tag="sdd")
                n_t = ins.tile([P, G, 3, S], F32, tag="n")
                nc.sync.dma_start(out=crn_t, in_=crn_r[t])
                nc.sync.dma_start(out=sdd_t, in_=sdd_r[t])
                nc.sync.dma_start(out=n_t, in_=n_r[t])
                s_t = sdd_t[:, :, 0:S]
                d_t = sdd_t[:, :, S:2 * S]
                dir_t = sdd_t[:, :, 2 * S:2 * S + 3]
                c_t = crn_t[:, :, 0:3, :]
                rn_t = crn_t[:, :, 3:6, :]

                # ---- GPSIMD front chain (feed-forward, DMA-fed only) ----
                # q[s] = dot(dir, n[s]);  tc2 = q[s]+q[s+1] = 2*cos_mid
                nd = tmp.tile([P, G, 3, S], F32, tag="nd")
                for ch in range(3):
                    dir_b = dir_t[:, :, ch].unsqueeze(2).to_broadcast([P, G, S])
                    nc.gpsimd.tensor_tensor(nd[:, :, ch, :], n_t[:, :, ch, :],
                                            dir_b, ALU.mult)
                q = tmp.tile([P, G, S], F32, tag="q")
                nc.gpsimd.tensor_tensor(q, nd[:, :, 0, :], nd[:, :, 1, :], ALU.add)
                nc.gpsimd.tensor_tensor(q, q, nd[:, :, 2, :], ALU.add)
                tc2 = tmp.tile([P, G, SM], F32, tag="tc2")
                nc.gpsimd.tensor_tensor(tc2, q[:, :, 0:SM], q[:, :, 1:S], ALU.add)
                # m = relu(-0.5*tc2) = -iter_cos  (relu is positive-homogeneous)
                m = tmp.tile([P, G, SM], F32, tag="m")
                nc.scalar.activation(m, tc2, ACT.Relu, scale=-0.5)

                # deltas & 2*est sdfs: E+- = A2 +- m*delta
                dl = tmp.tile([P, G, SM], F32, tag="dl")
                nc.gpsimd.tensor_tensor(dl, d_t[:, :, 1:S], d_t[:, :, 0:SM], ALU.subtract)
                a2 = tmp.tile([P, G, SM], F32, tag="a2")
                nc.gpsimd.tensor_tensor(a2, s_t[:, :, 0:SM], s_t[:, :, 1:S], ALU.add)
                pp = tmp.tile([P, G, SM], F32, tag="pp")
                nc.gpsimd.tensor_tensor(pp, m, dl, ALU.mult)
                e2 = tmp.tile([P, G, 2, SM], F32, tag="e2")
                nc.gpsimd.tensor_tensor(e2[:, :, 0, :], a2, pp, ALU.add)       # 2*est_prev
                nc.gpsimd.tensor_tensor(e2[:, :, 1, :], a2, pp, ALU.subtract)  # 2*est_next
                cdf = tmp.tile([P, G, 2, SM], F32, tag="cdf")
                nc.scalar.activation(cdf, e2, ACT.Sigmoid, scale=float(k_half))

                # ---- alpha = clip((prev-next+1e-5)/(prev+1e-5), 0, 1) ----
                num2 = tmp.tile([P, G, SM], F32, tag="num2")
                nc.vector.scalar_tensor_tensor(num2, cdf[:, :, 0, :], 1e-5,
                                               cdf[:, :, 1, :], ALU.add, ALU.subtract)
                den = tmp.tile([P, G, SM], F32, tag="den")
                nc.scalar.activation(den, cdf[:, :, 0, :], ACT.Relu, bias=eps_ap)
                rden = tmp.tile([P, G, SM], F32, tag="rden")
                nc.vector.reciprocal_approx_fast(rden, den)
                alpha = tmp.tile([P, G, SM], F32, tag="alpha")
                nc.vector.tensor_tensor(alpha, num2, rden, ALU.mult)
                nc.vector.tensor_scalar(alpha, alpha, 1.0, 0.0, ALU.min, ALU.max)

                # ---- transmittance via ONE segmented scan over all groups:
                # z[g,0]=0, z[g,s]=om[g,s-1];  state = max(z*state, bmask)
                # bmask=1 at s==0 resets each segment exactly to 1.0f.
                z = tmp.tile([P, G, S], F32, tag="z")
                nc.vector.memset(z[:, :, 0:1], 0.0)
                nc.vector.tensor_scalar(z[:, :, 1:S], alpha, -1.0, 1.0000000001,
                                        ALU.mult, ALU.add)
                tx = tmp.tile([P, G, S], F32, tag="tx")
                nc.vector.tensor_tensor_scan(
                    tx.rearrange("p g s -> p (g s)"),
                    z.rearrange("p g s -> p (g s)"),
                    bmask.rearrange("p g s -> p (g s)"),
                    0.0, ALU.mult, ALU.max)

                # ---- w = alpha * tx;  wf padded with zeros at both ends
                wf = tmp.tile([P, G, SM + 2], F32, tag="wf")  # [0]=0, [1..47]=w, [48]=0
                nc.vector.memset(wf[:, :, 0:1], 0.0)
                nc.vector.memset(wf[:, :, SM + 1:SM + 2], 0.0)
                nc.vector.tensor_tensor(wf[:, :, 1:SM + 1], alpha, tx[:, :, 0:SM], ALU.mult)
                nc.sync.dma_start(out=w_r[t], in_=wf[:, :, 1:SM + 1])

                # ---- v[s] = w[s-1]+w[s]: composite = 0.5*sum v*x; sum v = 2*wt
                vv = tmp.tile([P, G, 2, S], F32, tag="vv")  # [0]=v, [1]=v*d
                v = vv[:, :, 0, :]
                nc.vector.tensor_tensor(v, wf[:, :, 0:S], wf[:, :, 1:S + 1], ALU.add)
                v_b = tmp.tile([P, G, S], BF16, tag="v_b")
                nc.vector.tensor_copy(v_b, v)
                del v_b

                vcn = tmp.tile([P, G, 6, S], BF16, tag="vcn")
                for ch in range(6):
                    nc.vector.tensor_tensor(vcn[:, :, ch, :], crn_t[:, :, ch, :],
                                            v_b, ALU.mult)
                nc.vector.tensor_tensor(vv[:, :, 1, :], d_t, v, ALU.mult)

                cns = tmp.tile([P, G, 6], F32, tag="cns")
                wd = tmp.tile([P, G, 2], F32, tag="wd")  # [0]=2*wt, [1]=dsum
                nc.vector.tensor_reduce(cns, vcn, mybir.AxisListType.X, ALU.add)
                nc.vector.tensor_reduce(wd, vv, mybir.AxisListType.X, ALU.add)
                wt2 = wd[:, :, 0]
                dsum = wd[:, :, 1]
                rgbs = cns[:, :, 0:3]
                nrms = cns[:, :, 3:6]

                # rwt2 = 1/(2*wt); wt2 in (~1e-5, 2], inside approx-fast range
                rwt2 = tmp.tile([P, G], F32, tag="rwt2")
                nc.vector.reciprocal_approx_fast(rwt2, wt2)

                # combo out: rgb (0:3) + depth (3) + normal (4:7)
                o_t = outs.tile([P, G, 7], F32, tag="o_t")
                nc.scalar.mul(o_t[:, :, 0:3], rgbs, 0.5)
                nc.vector.tensor_tensor(o_t[:, :, 3:4], dsum.unsqueeze(2),
                                        rwt2.unsqueeze(2), ALU.mult)
                rwt2_b = rwt2.unsqueeze(2).to_broadcast([P, G, 3])
                nc.vector.tensor_tensor(o_t[:, :, 4:7], nrms, rwt2_b, ALU.mult)
                nc.sync.dma_start(out=o_r[t], in_=o_t)

    nc.compile()
    return nc


_PROGRAM_CACHE: dict[float, bass.Bass] = {}


def _get_program(k_half: float) -> bass.Bass:
    if k_half not in _PROGRAM_CACHE:
        _PROGRAM_CACHE[k_half] = _build_program(k_half)
    return _PROGRAM_CACHE[k_half]


def kernel(colors, sdfs, depths, normals, ray_directions, real_normals,
           inv_std_param, _trace=False):
    colors = np.asarray(colors, dtype=np.float32)
    sdfs = np.asarray(sdfs, dtype=np.float32)
    depths = np.asarray(depths, dtype=np.float32)
    normals = np.asarray(normals, dtype=np.float32)
    ray_directions = np.asarray(ray_directions, dtype=np.float32)
    real_normals = np.asarray(real_normals, dtype=np.float32)

    p = np.float32(np.asarray(inv_std_param).reshape(()))
    inv_std = np.clip(np.exp(np.float32(10.0) * p), np.float32(1e-6), np.float32(1e6))
    k_half = float(np.float32(inv_std) * np.float32(0.5))

    nc = _get_program(k_half)

    # host prep (layout/dtype only): flatten rays, ch-major bf16, concat
    crn = np.empty((RAYS, 6, S), dtype=BF)
    crn[:, 0:3] = colors.reshape(RAYS, S, 3).transpose(0, 2, 1)
    crn[:, 3:6] = real_normals.reshape(RAYS, S, 3).transpose(0, 2, 1)
    sdd = np.empty((RAYS, 2 * S + 3), dtype=np.float32)
    sdd[:, 0:S] = sdfs.reshape(RAYS, S)
    sdd[:, S:2 * S] = depths.reshape(RAYS, S)
    sdd[:, 2 * S:] = ray_directions.reshape(RAYS, 3)
    nf = np.ascontiguousarray(normals.reshape(RAYS, S, 3).transpose(0, 2, 1))

    in_maps = []
    for k in range(N_CORES):
        lo, hi = k * RAYS_PER_CORE, (k + 1) * RAYS_PER_CORE
        in_maps.append({
            "crn": crn[lo:hi],
            "sdd": sdd[lo:hi],
            "normals": nf[lo:hi],
        })

    res = bass_utils.run_bass_kernel_spmd(
        nc, in_maps, core_ids=list(range(N_CORES)), trace=_trace)

    w = np.concatenate([res.results[k]["wout"] for k in range(N_CORES)], axis=0)
    combo = np.concatenate([res.results[k]["combo"] for k in range(N_CORES)], axis=0)
    rgb, dep, nrm = combo[:, 0:3], combo[:, 3], combo[:, 4:7]

    # faithful edge handling (no-ops for non-degenerate rays)
    dep = np.nan_to_num(dep, nan=np.inf)
    dep = np.clip(dep, depths.min(), depths.max())
    nrm = np.nan_to_num(nrm, nan=np.inf)
    nrm = np.clip(nrm, real_normals.min(), real_normals.max())

    out = (np.ascontiguousarray(rgb).reshape(B, R, 3).astype(np.float32),
           dep.reshape(B, R, 1).astype(np.float32),
           w.reshape(B, R, SM, 1).astype(np.float32),
           np.ascontiguousarray(nrm).reshape(B, R, 3).astype(np.float32))
    if _trace:
        return out, res
    return out
